# revision 28
# baseline (speedup 1.0000x reference)
"""Trainium2 Bass kernel for nn_DeepTransitionRNN_31928786878509.

kernel(**inputs) -> np.ndarray, matching reference.reference semantics:
a deep-transition GRU over T=512 steps, B=128 (packed-sequence masking),
D=H=256, L=4 transition layers.

Strategy: data-parallel over batch (16 rows/core on 8 cores). Each core runs
the full 512-step recurrence with h resident in SBUF as fp16 triplets,
weights stationary as fp16 [128,128] chunks, PSUM fp32 preacts.

The per-step serial chain runs almost entirely on the vector engine via
custom DVE ops (polynomial sigma/tanh fused with the gating multiplies), so
each sub-layer is MM -> SIG -> TANHMUL -> blend-scan with no scalar-engine
round trips on the critical path. The scalar engine computes the exact
z-gate sigmoids and tanh(Cx@x) off-chain; gpsimd builds the scan operand
tiles. The x-projection matmuls for step t+1 are emitted during step t so
the PE never stalls the chain on them.

Cell tanh(v) (range +-4.6, too wide for a short poly) is handled by a
first-order Taylor expansion around TC=tanh(Cx@x) (exact, scalar engine,
off-chain): tanh(TC_pre + u) ~= TC + u*(1-TC^2), |u|<=0.15 so the residual
is ~1e-3. Packed-sequence masking (out=0 for t >= lengths[b]) is applied on
the host; inactive rows free-run on device (row-independent recurrence).

End-to-end numeric sim of this exact pipeline: rel err 2.6e-3 (tol 2e-2).
"""

import os
import numpy as np
from contextlib import ExitStack

import concourse.bass as bass
import concourse.bacc as bacc
import concourse.mybir as mybir
import concourse.tile as tile
from concourse.bass_utils import run_bass_kernel_spmd

f16 = mybir.dt.float16
f32 = mybir.dt.float32
AF = mybir.ActivationFunctionType
OP = mybir.AluOpType

T, B, D, H, L = 512, 128, 256, 256, 4
NCORE = 8
BS = B // NCORE
KC_D = D // 128
KC_H = H // 128
MC = H // 128
NCH = 3 * 4 * MC + 3 * KC_H * MC + 3 * L * KC_H * MC  # 84

UNROLL = 8

LAST_EXEC_NS = None  # set by kernel() when tracing is enabled

# Odd-poly coefficients (x * P(x^2) form), Chebyshev near-minimax fits.
# sigma(x)-0.5 on [-2.2, 2.2] deg5 (transition r-gate; |preact| <= 1.76)
C_SIG_T = (0.24963752412990847, -0.019365899964941327, 0.0011236406510448827)
# tanh(y) on [-1.45, 1.45] deg5 (transition candidate; |y| <= 1.09)
C_TANH_T = (0.9949740753086165, -0.2839567352534145, 0.05055024429496054)
# sigma(x)-0.5 on [-6.4, 6.4] deg7 (cell r-gate; |preact| <= 4.92; error is
# further scaled by |Ch@h| <= 0.15 downstream)
C_SIG_C = (0.2433623276813154, -0.013536065246952268, 0.0004300725682218298,
           -4.88256747468386e-06)

_DVE_OPS = {}


def _register_custom_ops():
    """Register the fused poly ops into concourse.dve_ops (idempotent)."""
    if _DVE_OPS:
        return _DVE_OPS
    from concourse.dve_ops import (DveOp, OPS, CUSTOM_DVE_SPECS,
                                   _SUB_OPCODE_FOR_NAME, _CUSTOM_DVE_ROW_BASE)
    from concourse.dve_spec import (Spec, Src0, Src1, C0, C1, C2, C3, sq,
                                    lower, _spill_c3_to_src1)
    from concourse.dve_uop import DveOpSpec

    existing = {op.name: op for op in OPS}

    def reg(name, spec):
        if name in existing:
            _DVE_OPS[name] = existing[name]
            return
        row = _CUSTOM_DVE_ROW_BASE + len(OPS)
        assert row < 0x20
        _SUB_OPCODE_FOR_NAME[name] = row
        shas = {}
        for ver in ("v3", "v4"):
            uops = lower(spec, ver=ver)
            shas[ver] = DveOpSpec(name=name, opcode=row, uops=uops,
                                  rd1_en=True).sha(ver)
        op = DveOp(name, spec, False, shas)
        OPS.append(op)
        CUSTOM_DVE_SPECS[name] = spec
        _DVE_OPS[name] = op

    t = sq(Src0)
    # out = x*(c0 + t(c1 + t*c2)) + 0.5   (0.5 via C3 -> in1 [P,1])
    reg("ANT_SIG5", Spec(
        body=_spill_c3_to_src1(Src0 * (C0 + t * (C1 + t * C2)) + C3),
        reference=lambda in0, in1, s0, s1, imm2: (
            in0.astype(np.float32) * (s0 + in0 * in0 * (s1 + in0 * in0 * imm2))
            + in1).astype(np.float32)))
    y = Src0 * Src1
    u = sq(y)
    # out = tanh5(Src0 * Src1)
    reg("ANT_TANHMUL5", Spec(
        body=y * (C0 + u * (C1 + u * C2)),
        reference=lambda in0, in1, s0, s1, imm2: (
            (in0 * in1).astype(np.float32)
            * (s0 + (in0 * in1) ** 2 * (s1 + (in0 * in1) ** 2 * imm2))
        ).astype(np.float32)))
    t7 = sq(Src0)
    # out = x*(c0 + t(c1 + t(c2 + t*c3)))  == sigma(x)-0.5, c3 via in1
    reg("ANT_SIG7CORE", Spec(
        body=_spill_c3_to_src1(Src0 * (C0 + t7 * (C1 + t7 * (C2 + t7 * C3)))),
        reference=lambda in0, in1, s0, s1, imm2: (
            in0.astype(np.float32)
            * (s0 + in0 ** 2 * (s1 + in0 ** 2 * (imm2 + in0 ** 2 * in1)))
        ).astype(np.float32)))
    return _DVE_OPS


def _pack_weights(Wr, Wz, Wl, Wt, Cx, Ch, Tr, Tz, Tn):
    chunks = []

    def add(M):
        for kc in range(M.shape[0] // 128):
            for mc in range(MC):
                chunks.append(M[kc * 128:(kc + 1) * 128, mc * 128:(mc + 1) * 128])

    add(Wr); add(Wz); add(Wl); add(Cx); add(Wt); add(Ch)
    for i in range(L):
        add(Tr[i]); add(Tz[i]); add(Tn[i])
    arr = np.stack([np.asarray(c, dtype=np.float32) for c in chunks])
    arr = arr.transpose(1, 0, 2).astype(np.float16)
    ident = np.eye(128, dtype=np.float16)[:, None, :]
    arr = np.ascontiguousarray(np.concatenate([arr, ident], axis=1))
    return arr


def _pack_x_shard(x_shard):
    Tn_ = x_shard.shape[0]
    y = np.asarray(x_shard, dtype=np.float16).reshape(Tn_, BS, KC_D, 128)
    y = y.transpose(0, 3, 2, 1)
    return np.ascontiguousarray(y.reshape(Tn_, 128, KC_D * BS))


def _build_nc(Tsteps, unroll):
    assert Tsteps % unroll == 0
    ops = _register_custom_ops()
    SIG5 = ops["ANT_SIG5"]
    TANHMUL5 = ops["ANT_TANHMUL5"]
    SIG7CORE = ops["ANT_SIG7CORE"]

    nc = bacc.Bacc(None, target_bir_lowering=False, debug=False)
    xin = nc.dram_tensor('xt', [Tsteps, 128, KC_D * BS], f16, kind='ExternalInput')
    win = nc.dram_tensor('wp', [128, NCH + 1, 128], f16, kind='ExternalInput')
    oul = nc.dram_tensor('out', [Tsteps, BS, H], f16, kind='ExternalOutput')

    idx = {}
    pos = 0

    def reg(name, kt):
        nonlocal pos
        idx[name] = [[pos + kc * MC + mc for mc in range(MC)] for kc in range(kt)]
        pos += kt * MC

    reg('Wr', 4); reg('Wz', 4); reg('Wl', 4)
    reg('Cx', 2); reg('Wt', 2); reg('Ch', 2)
    for i in range(L):
        reg(f'Tr{i}', KC_H); reg(f'Tz{i}', KC_H); reg(f'Tn{i}', KC_H)
    assert pos == NCH
    ID_CHUNK = NCH

    with ExitStack() as ctx:
        tc = ctx.enter_context(tile.TileContext(nc))
        wpool = ctx.enter_context(tc.tile_pool(name='w', bufs=1))
        hpool = ctx.enter_context(tc.tile_pool(name='h', bufs=1))
        spool = ctx.enter_context(tc.tile_pool(name='s', bufs=2))
        xpool = ctx.enter_context(tc.tile_pool(name='x', bufs=2))
        opool = ctx.enter_context(tc.tile_pool(name='o', bufs=2))
        ps_c_pool = ctx.enter_context(tc.tile_pool(name='ps_c', bufs=2, space='PSUM'))
        ps_lay_pool = ctx.enter_context(tc.tile_pool(name='ps_lay', bufs=2, space='PSUM'))
        ps_o_pool = ctx.enter_context(tc.tile_pool(name='ps_o', bufs=2, space='PSUM'))

        W = wpool.tile([128, NCH + 1, 128], f16)
        nc.gpsimd.dma_start(W[:], win[:])

        # h lives as fp16 triplets [128, KC_H, BS, 3]; slot 1 carries h
        # (the blend scan writes [n, h', 1] per element). Ping-pongs per
        # sub-layer between HTa/HTb.
        HTa = hpool.tile([128, KC_H, BS, 3], f16, tag='hta')
        HTb = hpool.tile([128, KC_H, BS, 3], f16, tag='htb')
        nc.gpsimd.memset(HTa[:], 0.0)
        nc.gpsimd.memset(HTb[:], 0.0)

        # scan operand tiles (fp32, SBUF).
        #   layer d0 = T3 [nn, zn, 0]   d1 = A3 [0, zz*h, 1]
        #   cell  d0 = CC3 [dl, z, 0]   d1 = DD3 [w+TC, (1-z)*h, 1]
        # with ops (mult, add), init 1:
        #   c0: s = d0[0]*1 + d1[0]
        #   c1: s = d0[1]*s + d1[1]   -> h'
        #   c2: s = 0*s + 1 = 1       -> re-arm
        T3 = [hpool.tile([128, MC, BS, 3], f32, tag=f't3{i}', name=f't3{i}')
              for i in range(2)]
        A3 = [hpool.tile([128, MC, BS, 3], f32, tag=f'a3{i}', name=f'a3{i}')
              for i in range(2)]
        CC3 = [hpool.tile([128, MC, BS, 3], f32, tag=f'cc3{i}', name=f'cc3{i}')
               for i in range(2)]
        DD3 = [hpool.tile([128, MC, BS, 3], f32, tag=f'dd3{i}', name=f'dd3{i}')
               for i in range(2)]
        for i in range(2):
            nc.gpsimd.memset(T3[i][:], 0.0)
            nc.gpsimd.memset(A3[i][:], 0.0)
            nc.gpsimd.memset(A3[i][:, :, :, 2], 1.0)
            nc.gpsimd.memset(CC3[i][:], 0.0)
            nc.gpsimd.memset(DD3[i][:], 0.0)
            nc.gpsimd.memset(DD3[i][:, :, :, 2], 1.0)

        HALF = hpool.tile([128, 1], f32, tag='half')
        SC7 = hpool.tile([128, 1], f32, tag='sc7')
        nc.gpsimd.memset(HALF[:], 0.5)
        nc.gpsimd.memset(SC7[:], C_SIG_C[3])

        # off-chain fp32 SBUF scratch (rotating)
        TCt = [hpool.tile([128, MC, BS], f32, tag=f'tc{i}', name=f'tc{i}')
               for i in range(2)]
        Gt = [hpool.tile([128, MC, BS], f32, tag=f'g{i}', name=f'g{i}')
              for i in range(2)]
        WtxS = [hpool.tile([128, MC, BS], f32, tag=f'wtx{i}', name=f'wtx{i}')
                for i in range(2)]

        def flat(ap):
            return ap.rearrange('p c b j -> p (c b j)')

        def f2(ap):
            # rank-2 view for custom-DVE ops (TTSS struct keeps imm2)
            return ap.rearrange('p c b -> p (c b)')

        def mm(out_ap, name, kc, mc, rhs, start, stop):
            nc.tensor.matmul(out_ap, W[:, idx[name][kc][mc], :], rhs,
                             start=start, stop=stop)

        def h_ap(tile_, kc):
            return tile_[:, kc, :, 1]

        def emit_x_mms(xt, pb):
            """x-projections for one step into the combined cell PSUM tile.
            Slots: 0=r, 1=z, 2=l, 3=Cx@x, 4=Wt@x, 5=Ch@h. Gate slots (0-2)
            are left open (stop lands on the h-part next step); x-only slots
            (3, 4) start and stop here."""
            for mc in range(MC):
                for kc in range(KC_D):
                    mm(pb[:, 0, mc], 'Wr', kc, mc, xt[:, kc],
                       mc == 0 and kc == 0, False)
            for gi, g in ((1, 'Wz'), (2, 'Wl')):
                for mc in range(MC):
                    for kc in range(KC_D):
                        mm(pb[:, gi, mc], g, kc, mc, xt[:, kc], False, False)
            for mc in range(MC):
                for kc in range(KC_D):
                    mm(pb[:, 3, mc], 'Cx', kc, mc, xt[:, kc], False, False)
            for mc in range(MC):
                for kc in range(KC_D):
                    mm(pb[:, 4, mc], 'Wt', kc, mc, xt[:, kc], False, False)

        def new_cell_psum():
            return ps_c_pool.tile([128, 6, MC, BS], f32, tag='ps_c',
                                  name='ps_c')

        def emit_step(si, pb, ob_slice, hcur, hother):
            i2 = si % 2
            # --- cell ---------------------------------------------------
            # off-chain x-only ACT work (inputs ready since last step)
            nc.scalar.activation(TCt[i2][:], pb[:, 3], AF.Tanh)
            nc.scalar.activation(WtxS[i2][:], pb[:, 4], AF.Copy)
            # gpsimd: G = 1 - TC^2
            nc.gpsimd.tensor_tensor(Gt[i2][:], TCt[i2][:], TCt[i2][:], OP.mult)
            nc.gpsimd.tensor_scalar(Gt[i2][:], Gt[i2][:], -1.0, 1.0,
                                    OP.mult, OP.add)

            # cell h-matmuls (chain: Wr first, then Ch)
            for mc in range(MC):
                for kc in range(KC_H):
                    mm(pb[:, 0, mc], 'Wr', KC_D + kc, mc, h_ap(hcur, kc),
                       False, mc == MC - 1 and kc == KC_H - 1)
            for mc in range(MC):
                for kc in range(KC_H):
                    mm(pb[:, 5, mc], 'Ch', kc, mc, h_ap(hcur, kc),
                       False, False)
            for gi, g in ((1, 'Wz'), (2, 'Wl')):
                for mc in range(MC):
                    for kc in range(KC_H):
                        mm(pb[:, gi, mc], g, KC_D + kc, mc,
                           h_ap(hcur, kc), False,
                           gi == 2 and mc == MC - 1 and kc == KC_H - 1)

            # off-chain ACT: exact z/l sigmoids (and zn_ = 1 - z)
            nc.scalar.activation(CC3[i2][:, :, :, 1], pb[:, 1], AF.Sigmoid)
            znc = spool.tile([128, MC, BS], f32, tag='znc')
            nc.scalar.activation(znc[:], pb[:, 1], AF.Sigmoid, scale=-1.0)
            slt = spool.tile([128, MC, BS], f32, tag='slt')
            nc.scalar.activation(slt[:], pb[:, 2], AF.Sigmoid)
            # gpsimd: w = sl*Wtx ; DD3[0] = w + TC ; DD3[1] = (1-z)*h
            wtmp = spool.tile([128, MC, BS], f32, tag='wtmp')
            nc.gpsimd.tensor_tensor(wtmp[:], slt[:], WtxS[i2][:], OP.mult)
            nc.gpsimd.tensor_tensor(DD3[i2][:, :, :, 0], wtmp[:], TCt[i2][:],
                                    OP.add)
            nc.gpsimd.tensor_tensor(DD3[i2][:, :, :, 1], znc[:],
                                    hcur[:, :, :, 1], OP.mult)

            # DVE chain: sig7core -> u=(s+.5)*Chh -> dl=u*G -> blend scan
            scel = spool.tile([128, MC, BS], f32, tag='scel')
            nc.vector._custom_dve(SIG7CORE, out=f2(scel[:]), in0=f2(pb[:, 0]),
                                  in1=SC7[:], s0=C_SIG_C[0], s1=C_SIG_C[1],
                                  imm2=C_SIG_C[2])
            ucel = spool.tile([128, MC, BS], f32, tag='ucel')
            nc.vector.scalar_tensor_tensor(ucel[:], scel[:], 0.5, pb[:, 5],
                                           OP.add, OP.mult)
            nc.vector.tensor_tensor(CC3[i2][:, :, :, 0], ucel[:], Gt[i2][:],
                                    OP.mult)
            nc.vector.tensor_tensor_scan(
                flat(hother[:]), flat(CC3[i2][:]), flat(DD3[i2][:]), 1.0,
                OP.mult, OP.add)
            hcur, hother = hother, hcur

            # --- transition layers ---------------------------------------
            for li in range(L):
                l2 = li % 2
                ps = ps_lay_pool.tile([128, 3, MC, BS], f32, tag='ps_lay')
                # z first (feeds the off-chain zn/A' path), then r, then n
                for mc in range(MC):
                    for kc in range(KC_H):
                        mm(ps[:, 0, mc], f'Tz{li}', kc, mc, h_ap(hcur, kc),
                           mc == 0 and kc == 0, False)
                for mc in range(MC):
                    for kc in range(KC_H):
                        mm(ps[:, 1, mc], f'Tr{li}', kc, mc, h_ap(hcur, kc),
                           False, False)
                for mc in range(MC):
                    for kc in range(KC_H):
                        mm(ps[:, 2, mc], f'Tn{li}', kc, mc, h_ap(hcur, kc),
                           False, mc == MC - 1 and kc == KC_H - 1)
                # off-chain: zz = sigma(zp) (A-path first), zn = sigma(-zp)
                zzt = spool.tile([128, MC, BS], f32, tag='zzt')
                nc.scalar.activation(zzt[:], ps[:, 0], AF.Sigmoid)
                nc.scalar.activation(T3[l2][:, :, :, 1], ps[:, 0], AF.Sigmoid,
                                     scale=-1.0)
                nc.gpsimd.tensor_tensor(A3[l2][:, :, :, 1], zzt[:],
                                        hcur[:, :, :, 1], OP.mult)
                # DVE chain
                sl_ = spool.tile([128, MC, BS], f32, tag='sl_')
                nc.vector._custom_dve(SIG5, out=f2(sl_[:]), in0=f2(ps[:, 1]),
                                      in1=HALF[:], s0=C_SIG_T[0],
                                      s1=C_SIG_T[1], imm2=C_SIG_T[2])
                nc.vector._custom_dve(TANHMUL5, out=f2(T3[l2][:, :, :, 0]),
                                      in0=f2(sl_[:]), in1=f2(ps[:, 2]),
                                      s0=C_TANH_T[0], s1=C_TANH_T[1],
                                      imm2=C_TANH_T[2])
                nc.vector.tensor_tensor_scan(
                    flat(hother[:]), flat(T3[l2][:]), flat(A3[l2][:]), 1.0,
                    OP.mult, OP.add)
                hcur, hother = hother, hcur

            # --- output --------------------------------------------------
            ps_o = ps_o_pool.tile([BS, MC, 128], f16, tag='ps_o')
            for c in range(KC_H):
                nc.tensor.matmul(ps_o[:, c, :], h_ap(hcur, c), W[:, ID_CHUNK, :],
                                 is_transpose=True, start=c == 0, stop=c == KC_H - 1)
            nc.scalar.copy(ob_slice, ps_o[:].rearrange('p c f -> p (c f)'))
            return hcur, hother

        nblocks = Tsteps // unroll
        xt_tiles = [None, None]
        xt_tiles[0] = xpool.tile([128, unroll, KC_D * BS], f16, tag='xt',
                                 name='xt')
        nc.sync.dma_start(
            xt_tiles[0][:], xin[0:unroll].rearrange('u p f -> p u f'))

        hcur, hother = HTa, HTb
        pb_cur = new_cell_psum()
        # prologue: x-projections for step 0
        emit_x_mms(xt_tiles[0][:, 0].rearrange('p (c b) -> p c b', c=KC_D),
                   pb_cur)

        for tb in range(nblocks):
            kb = tb % 2
            if tb + 1 < nblocks:
                xt_tiles[1 - kb] = xpool.tile([128, unroll, KC_D * BS], f16,
                                              tag='xt', name='xt')
                nc.sync.dma_start(
                    xt_tiles[1 - kb][:],
                    xin[(tb + 1) * unroll:(tb + 2) * unroll]
                    .rearrange('u p f -> p u f'))
            ob_tile = opool.tile([BS, unroll, H], f16, tag='ob')
            for j in range(unroll):
                si = tb * unroll + j
                # hoist x-projections for step si+1
                if si + 1 < Tsteps:
                    pb_next = new_cell_psum()
                    if j + 1 < unroll:
                        xt_next = xt_tiles[kb][:, j + 1]
                    else:
                        xt_next = xt_tiles[1 - kb][:, 0]
                    emit_x_mms(xt_next.rearrange('p (c b) -> p c b', c=KC_D),
                               pb_next)
                else:
                    pb_next = None
                hcur, hother = emit_step(si, pb_cur, ob_tile[:, j], hcur,
                                         hother)
                pb_cur = pb_next
            nc.sync.dma_start(
                oul[tb * unroll:(tb + 1) * unroll].rearrange('u b h -> b u h'),
                ob_tile[:])

    nc.compile()
    return nc


def _install_ntff_hook_shim():
    """The agent image lacks ``antenv.axon_hooks``; recreate it and register
    trn_boot's ctypes NTFF hook so trace=True works. Returns True on
    success."""
    import sys
    import types
    try:
        import antenv.axon_hooks  # noqa: F401
        return True
    except ImportError:
        pass
    try:
        import antenv
        from trn_agent_boot.trn_boot import _ntff_profile_via_ctypes
        mod = types.ModuleType('antenv.axon_hooks')
        mod._hook = _ntff_profile_via_ctypes('/opt/axon/libaxon_pjrt.so')
        mod.get_axon_ntff_profile_hook = lambda: mod._hook
        mod.set_axon_ntff_profile_hook = lambda h: setattr(mod, '_hook', h)
        sys.modules['antenv.axon_hooks'] = mod
        antenv.axon_hooks = mod
        return True
    except Exception as e:  # degrade to no-trace
        print(f'ntff hook shim failed: {e}')
        return False


def kernel(x, lengths, Wr, Wz, Wl, Wt, Cx, Ch, Tr, Tz, Tn):
    global LAST_EXEC_NS
    x = np.asarray(x)
    lengths = np.asarray(lengths)

    wp = _pack_weights(Wr, Wz, Wl, Wt, Cx, Ch, Tr, Tz, Tn)
    nc = _build_nc(T, UNROLL)

    in_maps = []
    for k in range(NCORE):
        xs = x[:, k * BS:(k + 1) * BS, :]
        in_maps.append({'xt': _pack_x_shard(xs), 'wp': wp})

    trace = bool(int(os.environ.get('RNN_KERNEL_TRACE', '0')))
    if trace:
        trace = _install_ntff_hook_shim()
    res = run_bass_kernel_spmd(nc, in_maps, core_ids=list(range(NCORE)),
                               trace=trace)
    LAST_EXEC_NS = res.exec_time_ns

    out = np.empty((T, B, H), np.float32)
    for k in range(NCORE):
        out[:, k * BS:(k + 1) * BS, :] = np.asarray(
            res.results[k]['out'], np.float32)
    mask = np.arange(T)[:, None] < lengths[None, :]
    out *= mask[:, :, None].astype(np.float32)
    return out


# revision 33
# speedup vs baseline: 1.1981x; 1.1981x over previous
"""Trainium2 Bass kernel for nn_DeepTransitionRNN_31928786878509.

kernel(**inputs) -> np.ndarray, matching reference.reference semantics:
a deep-transition GRU over T=512 steps, B=128 (packed-sequence masking),
D=H=256, L=4 transition layers.

Strategy: data-parallel over batch (16 rows/core on 8 cores). Each core runs
the full 512-step recurrence with h resident in SBUF as fp16 triplets,
weights stationary as fp16 [128,128] chunks, PSUM fp32 preacts.

The per-step serial chain runs almost entirely on the vector engine via
custom DVE ops (polynomial sigma/tanh fused with the gating multiplies), so
each sub-layer is MM -> SIG -> TANHMUL -> blend-scan with no scalar-engine
round trips on the critical path. The scalar engine computes the exact
z-gate sigmoids and tanh(Cx@x) off-chain; gpsimd builds the scan operand
tiles. The x-projection matmuls for step t+1 are emitted during step t so
the PE never stalls the chain on them.

Cell tanh(v) (range +-4.6, too wide for a short poly) is handled by a
first-order Taylor expansion around TC=tanh(Cx@x) (exact, scalar engine,
off-chain): tanh(TC_pre + u) ~= TC + u*(1-TC^2), |u|<=0.15 so the residual
is ~1e-3. Packed-sequence masking (out=0 for t >= lengths[b]) is applied on
the host; inactive rows free-run on device (row-independent recurrence).

End-to-end numeric sim of this exact pipeline: rel err 2.6e-3 (tol 2e-2).
"""

import os
import numpy as np
from contextlib import ExitStack

import concourse.bass as bass
import concourse.bacc as bacc
import concourse.mybir as mybir
import concourse.tile as tile
from concourse.bass_utils import run_bass_kernel_spmd

f16 = mybir.dt.float16
f32 = mybir.dt.float32
AF = mybir.ActivationFunctionType
OP = mybir.AluOpType

T, B, D, H, L = 512, 128, 256, 256, 4
NCORE = 8
BS = B // NCORE
KC_D = D // 128
KC_H = H // 128
MC = H // 128
NCH = 3 * 4 * MC + 3 * KC_H * MC + 3 * L * KC_H * MC  # 84

UNROLL = 8

LAST_EXEC_NS = None  # set by kernel() when tracing is enabled

# Odd-poly coefficients (x * P(x^2) form), Chebyshev near-minimax fits.
# sigma(x)-0.5 on [-2.2, 2.2] deg5 (transition r-gate; |preact| <= 1.76)
C_SIG_T = (0.24963752412990847, -0.019365899964941327, 0.0011236406510448827)
# tanh(y) on [-1.45, 1.45] deg5 (transition candidate; |y| <= 1.09)
C_TANH_T = (0.9949740753086165, -0.2839567352534145, 0.05055024429496054)
# sigma(x)-0.5 on [-6.4, 6.4] deg7 (cell r-gate; |preact| <= 4.92; error is
# further scaled by |Ch@h| <= 0.15 downstream)
C_SIG_C = (0.2433623276813154, -0.013536065246952268, 0.0004300725682218298,
           -4.88256747468386e-06)

_DVE_OPS = {}


def _register_custom_ops():
    """Register the fused poly ops into concourse.dve_ops (idempotent)."""
    if _DVE_OPS:
        return _DVE_OPS
    from concourse.dve_ops import (DveOp, OPS, CUSTOM_DVE_SPECS,
                                   _SUB_OPCODE_FOR_NAME, _CUSTOM_DVE_ROW_BASE)
    from concourse.dve_spec import (Spec, Src0, Src1, C0, C1, C2, C3, sq,
                                    lower, _spill_c3_to_src1)
    from concourse.dve_uop import DveOpSpec

    existing = {op.name: op for op in OPS}

    def reg(name, spec):
        if name in existing:
            _DVE_OPS[name] = existing[name]
            return
        row = _CUSTOM_DVE_ROW_BASE + len(OPS)
        assert row < 0x20
        _SUB_OPCODE_FOR_NAME[name] = row
        shas = {}
        for ver in ("v3", "v4"):
            uops = lower(spec, ver=ver)
            shas[ver] = DveOpSpec(name=name, opcode=row, uops=uops,
                                  rd1_en=True).sha(ver)
        op = DveOp(name, spec, False, shas)
        OPS.append(op)
        CUSTOM_DVE_SPECS[name] = spec
        _DVE_OPS[name] = op

    t = sq(Src0)
    # out = x*(c0 + t(c1 + t*c2)) + 0.5   (0.5 via C3 -> in1 [P,1])
    reg("ANT_SIG5", Spec(
        body=_spill_c3_to_src1(Src0 * (C0 + t * (C1 + t * C2)) + C3),
        reference=lambda in0, in1, s0, s1, imm2: (
            in0.astype(np.float32) * (s0 + in0 * in0 * (s1 + in0 * in0 * imm2))
            + in1).astype(np.float32)))
    y = Src0 * Src1
    u = sq(y)
    # out = tanh5(Src0 * Src1)
    reg("ANT_TANHMUL5", Spec(
        body=y * (C0 + u * (C1 + u * C2)),
        reference=lambda in0, in1, s0, s1, imm2: (
            (in0 * in1).astype(np.float32)
            * (s0 + (in0 * in1) ** 2 * (s1 + (in0 * in1) ** 2 * imm2))
        ).astype(np.float32)))
    t7 = sq(Src0)
    # out = x*(c0 + t(c1 + t(c2 + t*c3)))  == sigma(x)-0.5, c3 via in1
    reg("ANT_SIG7CORE", Spec(
        body=_spill_c3_to_src1(Src0 * (C0 + t7 * (C1 + t7 * (C2 + t7 * C3)))),
        reference=lambda in0, in1, s0, s1, imm2: (
            in0.astype(np.float32)
            * (s0 + in0 ** 2 * (s1 + in0 ** 2 * (imm2 + in0 ** 2 * in1)))
        ).astype(np.float32)))
    return _DVE_OPS


def _pack_weights(Wr, Wz, Wl, Wt, Cx, Ch, Tr, Tz, Tn):
    chunks = []

    def add(M):
        for kc in range(M.shape[0] // 128):
            for mc in range(MC):
                chunks.append(M[kc * 128:(kc + 1) * 128, mc * 128:(mc + 1) * 128])

    add(Wr); add(Wz); add(Wl); add(Cx); add(Wt); add(Ch)
    for i in range(L):
        add(Tr[i]); add(Tz[i]); add(Tn[i])
    arr = np.stack([np.asarray(c, dtype=np.float32) for c in chunks])
    arr = arr.transpose(1, 0, 2).astype(np.float16)
    ident = np.eye(128, dtype=np.float16)[:, None, :]
    arr = np.ascontiguousarray(np.concatenate([arr, ident], axis=1))
    return arr


def _pack_x_shard(x_shard):
    Tn_ = x_shard.shape[0]
    y = np.asarray(x_shard, dtype=np.float16).reshape(Tn_, BS, KC_D, 128)
    y = y.transpose(0, 3, 2, 1)
    return np.ascontiguousarray(y.reshape(Tn_, 128, KC_D * BS))


def _build_nc(Tsteps, unroll):
    assert Tsteps % unroll == 0
    ops = _register_custom_ops()
    SIG5 = ops["ANT_SIG5"]
    TANHMUL5 = ops["ANT_TANHMUL5"]
    SIG7CORE = ops["ANT_SIG7CORE"]

    nc = bacc.Bacc(None, target_bir_lowering=False, debug=False)
    xin = nc.dram_tensor('xt', [Tsteps, 128, KC_D * BS], f16, kind='ExternalInput')
    win = nc.dram_tensor('wp', [128, NCH + 1, 128], f16, kind='ExternalInput')
    oul = nc.dram_tensor('out', [Tsteps, BS, H], f16, kind='ExternalOutput')

    idx = {}
    pos = 0

    def reg(name, kt):
        nonlocal pos
        idx[name] = [[pos + kc * MC + mc for mc in range(MC)] for kc in range(kt)]
        pos += kt * MC

    reg('Wr', 4); reg('Wz', 4); reg('Wl', 4)
    reg('Cx', 2); reg('Wt', 2); reg('Ch', 2)
    for i in range(L):
        reg(f'Tr{i}', KC_H); reg(f'Tz{i}', KC_H); reg(f'Tn{i}', KC_H)
    assert pos == NCH
    ID_CHUNK = NCH

    with ExitStack() as ctx:
        tc = ctx.enter_context(tile.TileContext(nc))
        wpool = ctx.enter_context(tc.tile_pool(name='w', bufs=1))
        hpool = ctx.enter_context(tc.tile_pool(name='h', bufs=1))
        spool = ctx.enter_context(tc.tile_pool(name='s', bufs=2))
        xpool = ctx.enter_context(tc.tile_pool(name='x', bufs=2))
        opool = ctx.enter_context(tc.tile_pool(name='o', bufs=2))
        ps_c_pool = ctx.enter_context(tc.tile_pool(name='ps_c', bufs=2, space='PSUM'))
        ps_lay_pool = ctx.enter_context(tc.tile_pool(name='ps_lay', bufs=2, space='PSUM'))
        ps_o_pool = ctx.enter_context(tc.tile_pool(name='ps_o', bufs=2, space='PSUM'))

        W = wpool.tile([128, NCH + 1, 128], f16)
        nc.gpsimd.dma_start(W[:], win[:])

        # h lives as fp16 triplets [128, KC_H, BS, 3]; slot 1 carries h
        # (the blend scan writes [n, h', 1] per element). Ping-pongs per
        # sub-layer between HTa/HTb.
        HTa = hpool.tile([128, KC_H, BS, 3], f16, tag='hta')
        HTb = hpool.tile([128, KC_H, BS, 3], f16, tag='htb')
        nc.gpsimd.memset(HTa[:], 0.0)
        nc.gpsimd.memset(HTb[:], 0.0)

        # scan operand tiles (fp32, SBUF).
        #   layer d0 = T3 [nn, zn, 0]   d1 = A3 [0, zz*h, 1]
        #   cell  d0 = CC3 [dl, z, 0]   d1 = DD3 [w+TC, (1-z)*h, 1]
        # with ops (mult, add), init 1:
        #   c0: s = d0[0]*1 + d1[0]
        #   c1: s = d0[1]*s + d1[1]   -> h'
        #   c2: s = 0*s + 1 = 1       -> re-arm
        T3 = [hpool.tile([128, MC, BS, 3], f32, tag=f't3{i}', name=f't3{i}')
              for i in range(2)]
        A3 = [hpool.tile([128, MC, BS, 3], f32, tag=f'a3{i}', name=f'a3{i}')
              for i in range(2)]
        CC3 = [hpool.tile([128, MC, BS, 3], f32, tag=f'cc3{i}', name=f'cc3{i}')
               for i in range(2)]
        DD3 = [hpool.tile([128, MC, BS, 3], f32, tag=f'dd3{i}', name=f'dd3{i}')
               for i in range(2)]
        for i in range(2):
            nc.gpsimd.memset(T3[i][:], 0.0)
            nc.gpsimd.memset(A3[i][:], 0.0)
            nc.gpsimd.memset(A3[i][:, :, :, 2], 1.0)
            nc.gpsimd.memset(CC3[i][:], 0.0)
            nc.gpsimd.memset(DD3[i][:], 0.0)
            nc.gpsimd.memset(DD3[i][:, :, :, 2], 1.0)

        HALF = hpool.tile([128, 1], f32, tag='half')
        SC7 = hpool.tile([128, 1], f32, tag='sc7')
        nc.gpsimd.memset(HALF[:], 0.5)
        nc.gpsimd.memset(SC7[:], C_SIG_C[3])

        # off-chain fp32 SBUF scratch (rotating)
        TCt = [hpool.tile([128, MC, BS], f32, tag=f'tc{i}', name=f'tc{i}')
               for i in range(2)]
        Gt = [hpool.tile([128, MC, BS], f32, tag=f'g{i}', name=f'g{i}')
              for i in range(2)]
        WtxS = [hpool.tile([128, MC, BS], f32, tag=f'wtx{i}', name=f'wtx{i}')
                for i in range(2)]

        def flat(ap):
            return ap.rearrange('p c b j -> p (c b j)')

        def f2(ap):
            # rank-2 view for custom-DVE ops (TTSS struct keeps imm2)
            return ap.rearrange('p c b -> p (c b)')

        def mm(out_ap, name, kc, mc, rhs, start, stop):
            nc.tensor.matmul(out_ap, W[:, idx[name][kc][mc], :], rhs,
                             start=start, stop=stop)

        def h_ap(tile_, kc):
            return tile_[:, kc, :, 1]

        def emit_x_gates(xt, pb):
            """x-projections of the r/z/l gates for one step into the
            combined cell PSUM tile. Slots: 0=r, 1=z, 2=l, 3=Cx@x, 4=Wt@x,
            5=Ch@h. Gate slots are left open (stop lands on the h-part next
            step). Emitted in the PE-idle tail after the cell's own critical
            matmuls so they never delay the chain."""
            for mc in range(MC):
                for kc in range(KC_D):
                    mm(pb[:, 0, mc], 'Wr', kc, mc, xt[:, kc],
                       mc == 0 and kc == 0, False)
            for gi, g in ((1, 'Wz'), (2, 'Wl')):
                for mc in range(MC):
                    for kc in range(KC_D):
                        mm(pb[:, gi, mc], g, kc, mc, xt[:, kc], False, False)

        def emit_x_cand(xt, pb):
            """Cx@x and Wt@x for one step (PE-idle tail of layer 0)."""
            for mc in range(MC):
                for kc in range(KC_D):
                    mm(pb[:, 3, mc], 'Cx', kc, mc, xt[:, kc], False, False)
            for mc in range(MC):
                for kc in range(KC_D):
                    mm(pb[:, 4, mc], 'Wt', kc, mc, xt[:, kc], False, False)

        def new_cell_psum():
            return ps_c_pool.tile([128, 6, MC, BS], f32, tag='ps_c',
                                  name='ps_c')

        def emit_step(si, pb, pb_next, xt_next, ob_slice, hcur, hother):
            i2 = si % 2
            # --- cell ---------------------------------------------------
            # off-chain x-only ACT work (inputs ready since last step)
            nc.scalar.activation(TCt[i2][:], pb[:, 3], AF.Tanh)
            nc.scalar.activation(WtxS[i2][:], pb[:, 4], AF.Copy)
            # gpsimd: G = 1 - TC^2
            nc.gpsimd.tensor_tensor(Gt[i2][:], TCt[i2][:], TCt[i2][:], OP.mult)
            nc.gpsimd.tensor_scalar(Gt[i2][:], Gt[i2][:], -1.0, 1.0,
                                    OP.mult, OP.add)

            # cell h-matmuls (chain: Wr first, then Ch)
            for mc in range(MC):
                for kc in range(KC_H):
                    mm(pb[:, 0, mc], 'Wr', KC_D + kc, mc, h_ap(hcur, kc),
                       False, mc == MC - 1 and kc == KC_H - 1)
            for mc in range(MC):
                for kc in range(KC_H):
                    mm(pb[:, 5, mc], 'Ch', kc, mc, h_ap(hcur, kc),
                       False, False)
            for gi, g in ((1, 'Wz'), (2, 'Wl')):
                for mc in range(MC):
                    for kc in range(KC_H):
                        mm(pb[:, gi, mc], g, KC_D + kc, mc,
                           h_ap(hcur, kc), False,
                           gi == 2 and mc == MC - 1 and kc == KC_H - 1)
            # next step's x gate projections ride the PE-idle cell tail
            if pb_next is not None:
                emit_x_gates(xt_next, pb_next)

            # off-chain ACT: exact z/l sigmoids (and zn_ = 1 - z)
            nc.scalar.activation(CC3[i2][:, :, :, 1], pb[:, 1], AF.Sigmoid)
            znc = spool.tile([128, MC, BS], f32, tag='znc')
            nc.scalar.activation(znc[:], pb[:, 1], AF.Sigmoid, scale=-1.0)
            slt = spool.tile([128, MC, BS], f32, tag='slt')
            nc.scalar.activation(slt[:], pb[:, 2], AF.Sigmoid)
            # gpsimd: w = sl*Wtx ; DD3[0] = w + TC ; DD3[1] = (1-z)*h
            wtmp = spool.tile([128, MC, BS], f32, tag='wtmp')
            nc.gpsimd.tensor_tensor(wtmp[:], slt[:], WtxS[i2][:], OP.mult)
            nc.gpsimd.tensor_tensor(DD3[i2][:, :, :, 0], wtmp[:], TCt[i2][:],
                                    OP.add)
            nc.gpsimd.tensor_tensor(DD3[i2][:, :, :, 1], znc[:],
                                    hcur[:, :, :, 1], OP.mult)

            # DVE chain: sig7core -> u=(s+.5)*Chh -> dl=u*G -> blend scan
            scel = spool.tile([128, MC, BS], f32, tag='scel')
            nc.vector._custom_dve(SIG7CORE, out=f2(scel[:]), in0=f2(pb[:, 0]),
                                  in1=SC7[:], s0=C_SIG_C[0], s1=C_SIG_C[1],
                                  imm2=C_SIG_C[2])
            ucel = spool.tile([128, MC, BS], f32, tag='ucel')
            nc.vector.scalar_tensor_tensor(ucel[:], scel[:], 0.5, pb[:, 5],
                                           OP.add, OP.mult)
            nc.vector.tensor_tensor(CC3[i2][:, :, :, 0], ucel[:], Gt[i2][:],
                                    OP.mult)
            nc.vector.tensor_tensor_scan(
                flat(hother[:]), flat(CC3[i2][:]), flat(DD3[i2][:]), 1.0,
                OP.mult, OP.add)
            hcur, hother = hother, hcur

            # --- transition layers ---------------------------------------
            for li in range(L):
                l2 = li % 2
                ps = ps_lay_pool.tile([128, 3, MC, BS], f32, tag='ps_lay')
                # z first (feeds the off-chain zn/A' path), then r, then n
                for mc in range(MC):
                    for kc in range(KC_H):
                        mm(ps[:, 0, mc], f'Tz{li}', kc, mc, h_ap(hcur, kc),
                           mc == 0 and kc == 0, False)
                for mc in range(MC):
                    for kc in range(KC_H):
                        mm(ps[:, 1, mc], f'Tr{li}', kc, mc, h_ap(hcur, kc),
                           False, False)
                for mc in range(MC):
                    for kc in range(KC_H):
                        mm(ps[:, 2, mc], f'Tn{li}', kc, mc, h_ap(hcur, kc),
                           False, mc == MC - 1 and kc == KC_H - 1)
                if li == 0 and pb_next is not None:
                    emit_x_cand(xt_next, pb_next)
                # off-chain: zz = sigma(zp) (A-path first), zn = sigma(-zp)
                zzt = spool.tile([128, MC, BS], f32, tag='zzt')
                nc.scalar.activation(zzt[:], ps[:, 0], AF.Sigmoid)
                nc.scalar.activation(T3[l2][:, :, :, 1], ps[:, 0], AF.Sigmoid,
                                     scale=-1.0)
                nc.gpsimd.tensor_tensor(A3[l2][:, :, :, 1], zzt[:],
                                        hcur[:, :, :, 1], OP.mult)
                # DVE chain
                sl_ = spool.tile([128, MC, BS], f32, tag='sl_')
                nc.vector._custom_dve(SIG5, out=f2(sl_[:]), in0=f2(ps[:, 1]),
                                      in1=HALF[:], s0=C_SIG_T[0],
                                      s1=C_SIG_T[1], imm2=C_SIG_T[2])
                nc.vector._custom_dve(TANHMUL5, out=f2(T3[l2][:, :, :, 0]),
                                      in0=f2(sl_[:]), in1=f2(ps[:, 2]),
                                      s0=C_TANH_T[0], s1=C_TANH_T[1],
                                      imm2=C_TANH_T[2])
                nc.vector.tensor_tensor_scan(
                    flat(hother[:]), flat(T3[l2][:]), flat(A3[l2][:]), 1.0,
                    OP.mult, OP.add)
                hcur, hother = hother, hcur

            # --- output --------------------------------------------------
            ps_o = ps_o_pool.tile([BS, MC, 128], f16, tag='ps_o')
            for c in range(KC_H):
                nc.tensor.matmul(ps_o[:, c, :], h_ap(hcur, c), W[:, ID_CHUNK, :],
                                 is_transpose=True, start=c == 0, stop=c == KC_H - 1)
            nc.scalar.copy(ob_slice, ps_o[:].rearrange('p c f -> p (c f)'))
            return hcur, hother

        nblocks = Tsteps // unroll
        xt_tiles = [None, None]
        xt_tiles[0] = xpool.tile([128, unroll, KC_D * BS], f16, tag='xt',
                                 name='xt')
        nc.sync.dma_start(
            xt_tiles[0][:], xin[0:unroll].rearrange('u p f -> p u f'))

        hcur, hother = HTa, HTb
        pb_cur = new_cell_psum()
        # prologue: x-projections for step 0
        x0 = xt_tiles[0][:, 0].rearrange('p (c b) -> p c b', c=KC_D)
        emit_x_gates(x0, pb_cur)
        emit_x_cand(x0, pb_cur)

        for tb in range(nblocks):
            kb = tb % 2
            if tb + 1 < nblocks:
                xt_tiles[1 - kb] = xpool.tile([128, unroll, KC_D * BS], f16,
                                              tag='xt', name='xt')
                nc.sync.dma_start(
                    xt_tiles[1 - kb][:],
                    xin[(tb + 1) * unroll:(tb + 2) * unroll]
                    .rearrange('u p f -> p u f'))
            ob_tile = opool.tile([BS, unroll, H], f16, tag='ob')
            for j in range(unroll):
                si = tb * unroll + j
                if si + 1 < Tsteps:
                    pb_next = new_cell_psum()
                    if j + 1 < unroll:
                        xt_next = xt_tiles[kb][:, j + 1]
                    else:
                        xt_next = xt_tiles[1 - kb][:, 0]
                    xt_next = xt_next.rearrange('p (c b) -> p c b', c=KC_D)
                else:
                    pb_next, xt_next = None, None
                hcur, hother = emit_step(si, pb_cur, pb_next, xt_next,
                                         ob_tile[:, j], hcur, hother)
                pb_cur = pb_next
            nc.sync.dma_start(
                oul[tb * unroll:(tb + 1) * unroll].rearrange('u b h -> b u h'),
                ob_tile[:])

    nc.compile()
    return nc


def _install_ntff_hook_shim():
    """The agent image lacks ``antenv.axon_hooks``; recreate it and register
    trn_boot's ctypes NTFF hook so trace=True works. Returns True on
    success."""
    import sys
    import types
    try:
        import antenv.axon_hooks  # noqa: F401
        return True
    except ImportError:
        pass
    try:
        import antenv
        from trn_agent_boot.trn_boot import _ntff_profile_via_ctypes
        mod = types.ModuleType('antenv.axon_hooks')
        mod._hook = _ntff_profile_via_ctypes('/opt/axon/libaxon_pjrt.so')
        mod.get_axon_ntff_profile_hook = lambda: mod._hook
        mod.set_axon_ntff_profile_hook = lambda h: setattr(mod, '_hook', h)
        sys.modules['antenv.axon_hooks'] = mod
        antenv.axon_hooks = mod
        return True
    except Exception as e:  # degrade to no-trace
        print(f'ntff hook shim failed: {e}')
        return False


def kernel(x, lengths, Wr, Wz, Wl, Wt, Cx, Ch, Tr, Tz, Tn):
    global LAST_EXEC_NS
    x = np.asarray(x)
    lengths = np.asarray(lengths)

    wp = _pack_weights(Wr, Wz, Wl, Wt, Cx, Ch, Tr, Tz, Tn)
    nc = _build_nc(T, UNROLL)

    in_maps = []
    for k in range(NCORE):
        xs = x[:, k * BS:(k + 1) * BS, :]
        in_maps.append({'xt': _pack_x_shard(xs), 'wp': wp})

    trace = bool(int(os.environ.get('RNN_KERNEL_TRACE', '0')))
    if trace:
        trace = _install_ntff_hook_shim()
    res = run_bass_kernel_spmd(nc, in_maps, core_ids=list(range(NCORE)),
                               trace=trace)
    LAST_EXEC_NS = res.exec_time_ns

    out = np.empty((T, B, H), np.float32)
    for k in range(NCORE):
        out[:, k * BS:(k + 1) * BS, :] = np.asarray(
            res.results[k]['out'], np.float32)
    mask = np.arange(T)[:, None] < lengths[None, :]
    out *= mask[:, :, None].astype(np.float32)
    return out
